# revision 6
# baseline (speedup 1.0000x reference)
"""Causal self-attention Trainium2 Bass kernel.

Problem: B=4, T=2048, C=1024, H=16 heads, head_dim=64, fp32.
    qkv = x @ Wqkv + bqkv ; per-head causal softmax attention ; out = attn @ Wo + bo

Sharding (8 NeuronCores): core c -> (batch b = c//2, head-group g = c%2).
Each core computes qkv for its batch restricted to its 8 heads, attention for
those heads, and a partial output projection against its 512 rows of Wo.
The host sums the two partials of each batch pair (the tensor-parallel
all-reduce) and stacks batches.

On-core dataflow (all matmuls in float32r mode, ~full PE rate, TF32-like
rounding of inputs; accumulation is fp32 in PSUM):
  Phase 1:  qT,kT [512hd x 2048t] and v [2048t x 512hd] from xT (host-side
            transpose of x[b]) and Wq/Wk/Wv column slices. Biases are applied
            as K=1 rank-1 matmul updates.
  Phase 2:  per (head, 512-query block): S_T[k,q] = kT^T-tiles @ qT, exp via
            ACT (scale=1/8 folded in; scores bounded so no max subtraction),
            causal masking via a host-provided triangular tile + sub-range
            accumulation, attnT_aug = [v|1]^T @ expS_T accumulated over key
            tiles (row 64 = softmax denominator), then normalize with a
            reciprocal + ones-column broadcast matmul.
  Phase 3:  out_partial[t,c] = attnT^T-tiles @ Wo-rows (+ bo on even cores).
"""

import sys

if "/opt/trn_rl_repo" not in sys.path:
    sys.path.insert(0, "/opt/trn_rl_repo")

import numpy as np

import concourse.bass as bass
import concourse.tile as tile
from concourse import bacc, mybir
from concourse.bass_utils import run_bass_kernel_spmd

F32 = mybir.dt.float32
F32R = mybir.dt.float32r
EXP = mybir.ActivationFunctionType.Exp

B, T, C = 4, 2048, 1024
H, D = 16, 64
HPC = 8          # heads per core
HD = HPC * D     # 512: per-core head-dim slab
N_CORES = 8
SCALE = D ** -0.5


def build_nc():
    nc = bacc.Bacc("TRN2", target_bir_lowering=False, debug=False)

    xT = nc.dram_tensor("xT", [C, T], F32R, kind="ExternalInput")
    wq = nc.dram_tensor("wq", [C, HD], F32R, kind="ExternalInput")
    wk = nc.dram_tensor("wk", [C, HD], F32R, kind="ExternalInput")
    wv = nc.dram_tensor("wv", [C, HD], F32R, kind="ExternalInput")
    wo = nc.dram_tensor("wo", [HD, C], F32R, kind="ExternalInput")
    bq = nc.dram_tensor("bq", [1, HD], F32R, kind="ExternalInput")
    bk = nc.dram_tensor("bk", [1, HD], F32R, kind="ExternalInput")
    bv = nc.dram_tensor("bv", [1, HD], F32R, kind="ExternalInput")
    bo = nc.dram_tensor("bo", [1, C], F32R, kind="ExternalInput")
    tri = nc.dram_tensor("tri", [128, 128], F32R, kind="ExternalInput")
    out = nc.dram_tensor("out", [T, C], F32, kind="ExternalOutput")

    KO = C // 128        # 8 contraction tiles over C
    TC = T // 512        # 4 t-chunks of 512
    NQ = T // 512        # 4 query blocks per head
    NKT = T // 128       # 16 key tiles
    HDO = HD // 128      # 4 hd tiles

    with tile.TileContext(nc) as tc:
        const = tc.alloc_tile_pool(name="const", bufs=1)
        persist = tc.alloc_tile_pool(name="persist", bufs=1)
        psum = tc.alloc_tile_pool(name="psum", bufs=3, space="PSUM")
        psum_aug = tc.alloc_tile_pool(name="psum_aug", bufs=2, space="PSUM")
        psum_bc = tc.alloc_tile_pool(name="psum_bc", bufs=2, space="PSUM")

        # --- constants ---
        ones_f = const.tile([1, 512], F32)
        ones_r = const.tile([1, 512], F32R)
        nc.vector.memset(ones_f[:], 1.0)
        nc.vector.tensor_copy(ones_r[:], ones_f[:])
        ones_col_f = const.tile([128, 1], F32)
        nc.vector.memset(ones_col_f[:], 1.0)
        tri_sb = const.tile([128, 128], F32R)
        nc.sync.dma_start(tri_sb[:], tri[:, :])
        bq_sb = const.tile([1, HD], F32R)
        bk_sb = const.tile([1, HD], F32R)
        bv_sb = const.tile([1, HD], F32R)
        bo_sb = const.tile([1, C], F32R)
        nc.sync.dma_start(bq_sb[:], bq[:, :])
        nc.sync.dma_start(bk_sb[:], bk[:, :])
        nc.sync.dma_start(bv_sb[:], bv[:, :])
        nc.sync.dma_start(bo_sb[:], bo[:, :])

        # --- persistent tensors ---
        qT_sb = persist.tile([128, HDO, T], F32R)    # [colpart, hd-outer, t]
        kT_sb = persist.tile([128, HDO, T], F32R)
        v_sb = persist.tile([128, NKT, HPC, D + 1], F32R)  # [tpart, ktile, head, d|1]
        nc.vector.tensor_copy(
            v_sb[:, :, :, D], ones_col_f[:, 0:1].to_broadcast([128, NKT, HPC])
        )

        # ---------------- Phase 1: qT, kT, v ----------------
        ph1 = tc.alloc_tile_pool(name="ph1", bufs=1)
        wq_sb = ph1.tile([128, KO, HD], F32R)
        wk_sb = ph1.tile([128, KO, HD], F32R)
        wv_sb = ph1.tile([128, KO, HD], F32R)
        nc.sync.dma_start(wq_sb[:], wq.rearrange("(ko p) m -> p ko m", p=128))
        nc.sync.dma_start(wk_sb[:], wk.rearrange("(ko p) m -> p ko m", p=128))
        nc.sync.dma_start(wv_sb[:], wv.rearrange("(ko p) m -> p ko m", p=128))
        xt_pool = tc.alloc_tile_pool(name="xt", bufs=9)

        for tc4 in range(TC):
            ts_ = slice(tc4 * 512, (tc4 + 1) * 512)
            xt = []
            for ko in range(KO):
                t_ = xt_pool.tile([128, 512], F32R, tag="xt")
                nc.sync.dma_start(t_[:], xT[ko * 128 : (ko + 1) * 128, ts_])
                xt.append(t_)
            # qT / kT column tiles: psum[colpart, t]
            for w_sb, b_sb, dst in ((wq_sb, bq_sb, qT_sb), (wk_sb, bk_sb, kT_sb)):
                for i in range(HDO):
                    cs = slice(i * 128, (i + 1) * 128)
                    ps = psum.tile([128, 512], F32, tag="mm")
                    for ko in range(KO):
                        nc.tensor.matmul(
                            ps[:], w_sb[:, ko, cs], xt[ko][:],
                            start=(ko == 0), stop=False,
                        )
                    nc.tensor.matmul(
                        ps[:], b_sb[0:1, cs], ones_r[0:1, :],
                        start=False, stop=True, skip_group_check=True,
                    )
                    nc.scalar.copy(dst[:, i, ts_], ps[:])
            # v tiles: psum[tpart, hd]
            for s in range(4):
                kt = tc4 * 4 + s
                ps = psum.tile([128, 512], F32, tag="mm")
                for ko in range(KO):
                    nc.tensor.matmul(
                        ps[:], xt[ko][:, s * 128 : (s + 1) * 128], wv_sb[:, ko, :],
                        start=(ko == 0), stop=False,
                    )
                nc.tensor.matmul(
                    ps[:], ones_r[0:1, 0:128], bv_sb[0:1, :],
                    start=False, stop=True, skip_group_check=True,
                )
                nc.vector.tensor_copy(
                    v_sb[:, kt, :, 0:D], ps[:].rearrange("p (h d) -> p h d", h=HPC)
                )

        xt_pool.release()
        ph1.release()

        # ---------------- Phase 2: attention ----------------
        ph2 = tc.alloc_tile_pool(name="ph2", bufs=1)
        attnT_sb = ph2.tile([128, HDO, T], F32R)
        e_pool = tc.alloc_tile_pool(name="e", bufs=4)
        r_pool = tc.alloc_tile_pool(name="r", bufs=3)

        for h in range(HPC):
            co, pr = h // 2, (h % 2) * 64
            for q in range(NQ):
                jmax = 4 * q + 3
                aug = psum_aug.tile([D + 1, 512], F32, tag="aug")
                for j in range(jmax + 1):
                    diag = j >= 4 * q
                    c0 = 128 * (j - 4 * q) if diag else 0
                    ncol = 512 - c0
                    ps = psum.tile([128, 512], F32, tag="mm")
                    nc.tensor.matmul(
                        ps[:, :ncol],
                        kT_sb[pr : pr + 64, co, j * 128 : (j + 1) * 128],
                        qT_sb[pr : pr + 64, co, q * 512 + c0 : (q + 1) * 512],
                        start=True, stop=True,
                    )
                    e = e_pool.tile([128, 512], F32R, tag="e")
                    nc.scalar.activation(e[:, :ncol], ps[:, :ncol], EXP, scale=SCALE)
                    if diag:
                        nc.vector.tensor_mul(e[:, 0:128], e[:, 0:128], tri_sb[:])
                    nc.tensor.matmul(
                        aug[:, c0:], v_sb[:, j, h, :], e[:, :ncol],
                        start=(j == 0), stop=(j == jmax), skip_group_check=True,
                    )
                recip = r_pool.tile([1, 512], F32R, tag="recip")
                with nc.allow_low_precision(reason="fp32r rounding of softmax denom"):
                    nc.vector.reciprocal(recip[:], aug[D : D + 1, :])
                bc = psum_bc.tile([64, 512], F32, tag="bc")
                nc.tensor.matmul(
                    bc[:], ones_r[0:1, 0:64], recip[:], start=True, stop=True
                )
                bc_sb = r_pool.tile([64, 512], F32R, tag="bc_sb")
                nc.scalar.copy(bc_sb[:], bc[:])
                nc.vector.tensor_mul(
                    attnT_sb[pr : pr + 64, co, q * 512 : (q + 1) * 512],
                    aug[0:D, :], bc_sb[:],
                )

        # ---------------- Phase 3: output projection ----------------
        ph3 = tc.alloc_tile_pool(name="ph3", bufs=1)
        wo_sb = ph3.tile([128, HDO, C], F32R)
        nc.sync.dma_start(wo_sb[:], wo.rearrange("(ko p) n -> p ko n", p=128))
        o_pool = tc.alloc_tile_pool(name="o", bufs=3)

        for tt in range(NKT):
            for cc in range(2):
                cs = slice(cc * 512, (cc + 1) * 512)
                ps = psum.tile([128, 512], F32, tag="mm")
                for ko in range(HDO):
                    nc.tensor.matmul(
                        ps[:], attnT_sb[:, ko, tt * 128 : (tt + 1) * 128],
                        wo_sb[:, ko, cs],
                        start=(ko == 0), stop=False,
                    )
                nc.tensor.matmul(
                    ps[:], ones_r[0:1, 0:128], bo_sb[0:1, cs],
                    start=False, stop=True, skip_group_check=True,
                )
                osb = o_pool.tile([128, 512], F32, tag="osb")
                nc.vector.tensor_copy(osb[:], ps[:])
                nc.sync.dma_start(out[tt * 128 : (tt + 1) * 128, cs], osb[:])

        o_pool.release()
        ph3.release()
        r_pool.release()
        e_pool.release()
        ph2.release()
        psum_bc.release()
        psum_aug.release()
        psum.release()
        persist.release()
        const.release()

    nc.finalize()
    return nc


_NC_CACHE = None


def _get_nc():
    global _NC_CACHE
    if _NC_CACHE is None:
        _NC_CACHE = build_nc()
    return _NC_CACHE


def make_in_maps(x, Wqkv, bqkv, Wo, bo):
    x = np.asarray(x, dtype=np.float32)
    Wqkv = np.asarray(Wqkv, dtype=np.float32)
    bqkv = np.asarray(bqkv, dtype=np.float32)
    Wo = np.asarray(Wo, dtype=np.float32)
    bo = np.asarray(bo, dtype=np.float32)

    w3 = Wqkv.reshape(C, 3, H, D)
    b3 = bqkv.reshape(3, H, D)
    wo4 = Wo.reshape(H, D, C)
    tri = np.triu(np.ones((128, 128), dtype=np.float32))

    in_maps = []
    for c in range(N_CORES):
        b, g = c // 2, c % 2
        hs = slice(g * HPC, (g + 1) * HPC)
        in_maps.append({
            "xT": np.ascontiguousarray(x[b].T),
            "wq": np.ascontiguousarray(w3[:, 0, hs, :].reshape(C, HD)),
            "wk": np.ascontiguousarray(w3[:, 1, hs, :].reshape(C, HD)),
            "wv": np.ascontiguousarray(w3[:, 2, hs, :].reshape(C, HD)),
            "wo": np.ascontiguousarray(wo4[hs].reshape(HD, C)),
            "bq": np.ascontiguousarray(b3[0, hs].reshape(1, HD)),
            "bk": np.ascontiguousarray(b3[1, hs].reshape(1, HD)),
            "bv": np.ascontiguousarray(b3[2, hs].reshape(1, HD)),
            # bo added once per batch pair (even core only)
            "bo": (bo.reshape(1, C).copy() if g == 0
                   else np.zeros((1, C), dtype=np.float32)),
            "tri": tri,
        })
    return in_maps


def run(x, Wqkv, bqkv, Wo, bo, **spmd_kwargs):
    nc = _get_nc()
    in_maps = make_in_maps(x, Wqkv, bqkv, Wo, bo)
    res = run_bass_kernel_spmd(nc, in_maps, core_ids=list(range(N_CORES)),
                               **spmd_kwargs)
    out = np.empty((B, T, C), dtype=np.float32)
    for b in range(B):
        out[b] = res.results[2 * b]["out"] + res.results[2 * b + 1]["out"]
    return out, res


def kernel(x, Wqkv, bqkv, Wo, bo):
    out, _ = run(x, Wqkv, bqkv, Wo, bo)
    return out


# revision 7
# speedup vs baseline: 1.1945x; 1.1945x over previous
"""Causal self-attention Trainium2 Bass kernel.

Problem: B=4, T=2048, C=1024, H=16 heads, head_dim=64, fp32.
    qkv = x @ Wqkv + bqkv ; per-head causal softmax attention ; out = attn @ Wo + bo

Sharding (8 NeuronCores): core c -> (batch b = c//2, head-group g = c%2).
Each core computes qkv for its batch restricted to its 8 heads, attention for
those heads, and a partial output projection against its 512 rows of Wo.
The host sums the two partials of each batch pair (the tensor-parallel
all-reduce), adds bo, and stacks batches.

On-core dataflow (matmul dtype MM_DT; PSUM accumulation is always fp32):
  Phase 1:  qT,kT [512hd x 2048t] and v [2048t x 512hd] from xT (host-side
            transpose of x[b]) and Wq/Wk/Wv column slices. bq/bk applied as
            per-partition adds during the PSUM->SBUF copy; bv as a K=1
            rank-1 matmul update.
  Phase 2:  per (head, 512-query block): S_T[k,q] = kT-tile^T @ qT, exp via
            ACT (scale=1/8 folded in; scores bounded ~|3.2| so no max
            subtraction), causal masking via a host triangular tile +
            sub-range accumulation, attnT_aug = [v|1]^T @ expS_T accumulated
            over key tiles (row 64 = softmax denominator).  Normalization is
            software-pipelined one block behind: denom row -> SBUF, ones x
            denom broadcast matmul -> PSUM, 64-lane reciprocal -> SBUF,
            multiply into attnT.
  Phase 3:  out_partial[t,c] = attnT-tile^T @ Wo-rows; bo added on host.
"""

import sys

if "/opt/trn_rl_repo" not in sys.path:
    sys.path.insert(0, "/opt/trn_rl_repo")

import numpy as np

import concourse.bass as bass
import concourse.tile as tile
from concourse import bacc, mybir
from concourse.bass_utils import run_bass_kernel_spmd

F32 = mybir.dt.float32
F32R = mybir.dt.float32r
F16 = mybir.dt.float16
BF16 = mybir.dt.bfloat16
EXP = mybir.ActivationFunctionType.Exp

# Matmul operand dtype: F32R (2 PE cycles/row, ~1.5e-4 matmul relerr) or
# F16/BF16 (1 cycle/row).
MM_DT = F32R

B, T, C = 4, 2048, 1024
H, D = 16, 64
HPC = 8          # heads per core
HD = HPC * D     # 512: per-core head-dim slab
N_CORES = 8
SCALE = D ** -0.5


def _np_of(dt):
    return np.dtype(mybir.dt.np(dt))


def build_nc(mm_dt=None):
    mm_dt = mm_dt or MM_DT
    nc = bacc.Bacc("TRN2", target_bir_lowering=False, debug=False)

    xT = nc.dram_tensor("xT", [C, T], mm_dt, kind="ExternalInput")
    wq = nc.dram_tensor("wq", [C, HD], mm_dt, kind="ExternalInput")
    wk = nc.dram_tensor("wk", [C, HD], mm_dt, kind="ExternalInput")
    wv = nc.dram_tensor("wv", [C, HD], mm_dt, kind="ExternalInput")
    wo = nc.dram_tensor("wo", [HD, C], mm_dt, kind="ExternalInput")
    # bq/bk as [128, HD//128] columns (per-partition adds in qkvT layout)
    bqc = nc.dram_tensor("bqc", [128, HD // 128], F32, kind="ExternalInput")
    bkc = nc.dram_tensor("bkc", [128, HD // 128], F32, kind="ExternalInput")
    bv = nc.dram_tensor("bv", [1, HD], mm_dt, kind="ExternalInput")
    tri = nc.dram_tensor("tri", [128, 128], mm_dt, kind="ExternalInput")
    out = nc.dram_tensor("out", [T, C], F32, kind="ExternalOutput")

    KO = C // 128        # 8 contraction tiles over C
    TC = T // 512        # 4 t-chunks of 512
    NQ = T // 512        # 4 query blocks per head
    NKT = T // 128       # 16 key tiles
    HDO = HD // 128      # 4 hd tiles

    with tile.TileContext(nc) as tc:
        const = tc.alloc_tile_pool(name="const", bufs=1)
        persist = tc.alloc_tile_pool(name="persist", bufs=1)
        psum = tc.alloc_tile_pool(name="psum", bufs=3, space="PSUM")
        psum_aug = tc.alloc_tile_pool(name="psum_aug", bufs=3, space="PSUM")
        psum_bc = tc.alloc_tile_pool(name="psum_bc", bufs=2, space="PSUM")

        # --- constants ---
        ones_f = const.tile([1, 512], F32)
        ones_r = const.tile([1, 512], mm_dt)
        nc.vector.memset(ones_f[:], 1.0)
        nc.vector.tensor_copy(ones_r[:], ones_f[:])
        ones_col_f = const.tile([128, 1], F32)
        nc.vector.memset(ones_col_f[:], 1.0)
        tri_sb = const.tile([128, 128], mm_dt)
        nc.sync.dma_start(tri_sb[:], tri[:, :])
        bqc_sb = const.tile([128, HD // 128], F32)
        bkc_sb = const.tile([128, HD // 128], F32)
        bv_sb = const.tile([1, HD], mm_dt)
        nc.sync.dma_start(bqc_sb[:], bqc[:, :])
        nc.sync.dma_start(bkc_sb[:], bkc[:, :])
        nc.sync.dma_start(bv_sb[:], bv[:, :])

        # --- persistent tensors ---
        qT_sb = persist.tile([128, HDO, T], mm_dt)   # [colpart, hd-outer, t]
        kT_sb = persist.tile([128, HDO, T], mm_dt)
        v_sb = persist.tile([128, NKT, HPC, D + 1], mm_dt)  # [tpart, ktile, head, d|1]
        nc.vector.tensor_copy(
            v_sb[:, :, :, D], ones_col_f[:, 0:1].to_broadcast([128, NKT, HPC])
        )

        # ---------------- Phase 1: qT, kT, v ----------------
        ph1 = tc.alloc_tile_pool(name="ph1", bufs=1)
        wq_sb = ph1.tile([128, KO, HD], mm_dt)
        wk_sb = ph1.tile([128, KO, HD], mm_dt)
        wv_sb = ph1.tile([128, KO, HD], mm_dt)
        nc.sync.dma_start(wq_sb[:], wq.rearrange("(ko p) m -> p ko m", p=128))
        nc.sync.dma_start(wk_sb[:], wk.rearrange("(ko p) m -> p ko m", p=128))
        nc.sync.dma_start(wv_sb[:], wv.rearrange("(ko p) m -> p ko m", p=128))
        xt_pool = tc.alloc_tile_pool(name="xt", bufs=9)

        for tc4 in range(TC):
            ts_ = slice(tc4 * 512, (tc4 + 1) * 512)
            xt = []
            for ko in range(KO):
                t_ = xt_pool.tile([128, 512], mm_dt, tag="xt")
                nc.sync.dma_start(t_[:], xT[ko * 128 : (ko + 1) * 128, ts_])
                xt.append(t_)
            # qT / kT column tiles: psum[colpart, t]; bias as per-partition add
            for w_sb, b_sb, dst in ((wq_sb, bqc_sb, qT_sb), (wk_sb, bkc_sb, kT_sb)):
                for i in range(HDO):
                    cs = slice(i * 128, (i + 1) * 128)
                    ps = psum.tile([128, 512], F32, tag="mm")
                    for ko in range(KO):
                        nc.tensor.matmul(
                            ps[:], w_sb[:, ko, cs], xt[ko][:],
                            start=(ko == 0), stop=(ko == KO - 1),
                        )
                    nc.vector.tensor_scalar_add(
                        dst[:, i, ts_], ps[:], b_sb[:, i : i + 1]
                    )
            # v tiles: psum[tpart, hd]; bias via K=1 rank-1 matmul
            for s in range(4):
                kt = tc4 * 4 + s
                ps = psum.tile([128, 512], F32, tag="mm")
                for ko in range(KO):
                    nc.tensor.matmul(
                        ps[:], xt[ko][:, s * 128 : (s + 1) * 128], wv_sb[:, ko, :],
                        start=(ko == 0), stop=False,
                    )
                nc.tensor.matmul(
                    ps[:], ones_r[0:1, 0:128], bv_sb[0:1, :],
                    start=False, stop=True, skip_group_check=True,
                )
                nc.scalar.copy(
                    v_sb[:, kt, :, 0:D], ps[:].rearrange("p (h d) -> p h d", h=HPC)
                )

        xt_pool.release()
        ph1.release()

        # ---------------- Phase 2: attention ----------------
        ph2 = tc.alloc_tile_pool(name="ph2", bufs=1)
        attnT_sb = ph2.tile([128, HDO, T], mm_dt)
        e_pool = tc.alloc_tile_pool(name="e", bufs=6)
        r_pool = tc.alloc_tile_pool(name="r", bufs=3)

        pending = None  # (aug, drow, pr, co, q) awaiting normalization

        def flush_norm():
            nonlocal pending
            if pending is None:
                return
            aug, drow, pr, co, q = pending
            bc = psum_bc.tile([64, 512], F32, tag="bc")
            nc.tensor.matmul(bc[:], ones_r[0:1, 0:64], drow[:], start=True, stop=True)
            rec = r_pool.tile([64, 512], F32, tag="rec")
            nc.vector.reciprocal(rec[:], bc[:])
            nc.vector.tensor_mul(
                attnT_sb[pr : pr + 64, co, q * 512 : (q + 1) * 512],
                aug[0:D, :], rec[:],
            )
            pending = None

        for h in range(HPC):
            co, pr = h // 2, (h % 2) * 64
            for q in range(NQ):
                jmax = 4 * q + 3
                aug = psum_aug.tile([D + 1, 512], F32, tag="aug")
                for j in range(jmax + 1):
                    diag = j >= 4 * q
                    c0 = 128 * (j - 4 * q) if diag else 0
                    ncol = 512 - c0
                    ps = psum.tile([128, 512], F32, tag="mm")
                    nc.tensor.matmul(
                        ps[:, :ncol],
                        kT_sb[pr : pr + 64, co, j * 128 : (j + 1) * 128],
                        qT_sb[pr : pr + 64, co, q * 512 + c0 : (q + 1) * 512],
                        start=True, stop=True,
                    )
                    e = e_pool.tile([128, 512], mm_dt, tag="e")
                    nc.scalar.activation(e[:, :ncol], ps[:, :ncol], EXP, scale=SCALE)
                    if diag:
                        nc.vector.tensor_mul(e[:, 0:128], e[:, 0:128], tri_sb[:])
                    nc.tensor.matmul(
                        aug[:, c0:], v_sb[:, j, h, :], e[:, :ncol],
                        start=(j == 0), stop=(j == jmax), skip_group_check=True,
                    )
                # denominator row out of PSUM, then normalize previous block
                drow = r_pool.tile([1, 512], mm_dt, tag="drow")
                with nc.allow_low_precision(reason="softmax denom rounding"):
                    nc.vector.tensor_copy(drow[:], aug[D : D + 1, :])
                flush_norm()
                pending = (aug, drow, pr, co, q)
        flush_norm()

        # ---------------- Phase 3: output projection ----------------
        ph3 = tc.alloc_tile_pool(name="ph3", bufs=1)
        wo_sb = ph3.tile([128, HDO, C], mm_dt)
        nc.sync.dma_start(wo_sb[:], wo.rearrange("(ko p) n -> p ko n", p=128))
        o_pool = tc.alloc_tile_pool(name="o", bufs=3)

        for tt in range(NKT):
            for cc in range(2):
                cs = slice(cc * 512, (cc + 1) * 512)
                ps = psum.tile([128, 512], F32, tag="mm")
                for ko in range(HDO):
                    nc.tensor.matmul(
                        ps[:], attnT_sb[:, ko, tt * 128 : (tt + 1) * 128],
                        wo_sb[:, ko, cs],
                        start=(ko == 0), stop=(ko == HDO - 1),
                    )
                osb = o_pool.tile([128, 512], F32, tag="osb")
                nc.vector.tensor_copy(osb[:], ps[:])
                nc.sync.dma_start(out[tt * 128 : (tt + 1) * 128, cs], osb[:])

        o_pool.release()
        ph3.release()
        r_pool.release()
        e_pool.release()
        ph2.release()
        psum_bc.release()
        psum_aug.release()
        psum.release()
        persist.release()
        const.release()

    nc.finalize()
    return nc


_NC_CACHE = {}


def _get_nc(mm_dt=None):
    key = str(mm_dt or MM_DT)
    if key not in _NC_CACHE:
        _NC_CACHE[key] = build_nc(mm_dt)
    return _NC_CACHE[key]


def make_in_maps(x, Wqkv, bqkv, Wo, mm_dt=None):
    mdt = _np_of(mm_dt or MM_DT)
    x = np.asarray(x, dtype=np.float32)
    Wqkv = np.asarray(Wqkv, dtype=np.float32)
    bqkv = np.asarray(bqkv, dtype=np.float32)
    Wo = np.asarray(Wo, dtype=np.float32)

    w3 = Wqkv.reshape(C, 3, H, D)
    b3 = bqkv.reshape(3, H, D)
    wo4 = Wo.reshape(H, D, C)
    tri = np.triu(np.ones((128, 128), dtype=np.float32))

    in_maps = []
    for c in range(N_CORES):
        b, g = c // 2, c % 2
        hs = slice(g * HPC, (g + 1) * HPC)
        bq = b3[0, hs].reshape(HD)
        bk = b3[1, hs].reshape(HD)
        in_maps.append({
            "xT": np.ascontiguousarray(x[b].T).astype(mdt),
            "wq": np.ascontiguousarray(w3[:, 0, hs, :].reshape(C, HD)).astype(mdt),
            "wk": np.ascontiguousarray(w3[:, 1, hs, :].reshape(C, HD)).astype(mdt),
            "wv": np.ascontiguousarray(w3[:, 2, hs, :].reshape(C, HD)).astype(mdt),
            "wo": np.ascontiguousarray(wo4[hs].reshape(HD, C)).astype(mdt),
            "bqc": np.ascontiguousarray(bq.reshape(HD // 128, 128).T).astype(np.float32),
            "bkc": np.ascontiguousarray(bk.reshape(HD // 128, 128).T).astype(np.float32),
            "bv": b3[2, hs].reshape(1, HD).astype(mdt),
            "tri": tri.astype(mdt),
        })
    return in_maps


def run(x, Wqkv, bqkv, Wo, bo, mm_dt=None, **spmd_kwargs):
    nc = _get_nc(mm_dt)
    in_maps = make_in_maps(x, Wqkv, bqkv, Wo, mm_dt=mm_dt)
    res = run_bass_kernel_spmd(nc, in_maps, core_ids=list(range(N_CORES)),
                               **spmd_kwargs)
    bo = np.asarray(bo, dtype=np.float32)
    out = np.empty((B, T, C), dtype=np.float32)
    for b in range(B):
        out[b] = res.results[2 * b]["out"] + res.results[2 * b + 1]["out"] + bo
    return out, res


def kernel(x, Wqkv, bqkv, Wo, bo):
    out, _ = run(x, Wqkv, bqkv, Wo, bo)
    return out


# revision 9
# speedup vs baseline: 1.3959x; 1.1686x over previous
"""Causal self-attention Trainium2 Bass kernel.

Problem: B=4, T=2048, C=1024, H=16 heads, head_dim=64, fp32.
    qkv = x @ Wqkv + bqkv ; per-head causal softmax attention ; out = attn @ Wo + bo

Sharding (8 NeuronCores): core c -> (batch b = c//2, head-group g = c%2).
Each core computes qkv for its batch restricted to its 8 heads, attention for
those heads, and a partial output projection against its 512 rows of Wo.
The host sums the two partials of each batch pair (the tensor-parallel
all-reduce), adds bo, and stacks batches.

On-core dataflow (matmul dtype MM_DT; PSUM accumulation is always fp32):
  Phase 1:  qT,kT [512hd x 2048t] and v [2048t x 512hd] from xT (host-side
            transpose of x[b]) and Wq/Wk/Wv column slices. bq/bk applied as
            per-partition adds during the PSUM->SBUF copy; bv as a K=1
            rank-1 matmul update.
  Phase 2:  per (head, 512-query block): S_T[k,q] = kT-tile^T @ qT, exp via
            ACT (scale=1/8 folded in; scores bounded ~|3.2| so no max
            subtraction), causal masking via a host triangular tile +
            sub-range accumulation, attnT_aug = [v|1]^T @ expS_T accumulated
            over key tiles (row 64 = softmax denominator).  Normalization is
            software-pipelined one block behind: denom row -> SBUF, ones x
            denom broadcast matmul -> PSUM, 64-lane reciprocal -> SBUF,
            multiply into attnT.
  Phase 3:  out_partial[t,c] = attnT-tile^T @ Wo-rows; bo added on host.
"""

import sys

if "/opt/trn_rl_repo" not in sys.path:
    sys.path.insert(0, "/opt/trn_rl_repo")

import numpy as np

import concourse.bass as bass
import concourse.tile as tile
from concourse import bacc, mybir
from concourse.bass_utils import run_bass_kernel_spmd

F32 = mybir.dt.float32
F32R = mybir.dt.float32r
F16 = mybir.dt.float16
BF16 = mybir.dt.bfloat16
EXP = mybir.ActivationFunctionType.Exp

# Matmul operand dtype: F32R (2 PE cycles/row, ~1.5e-4 matmul relerr) or
# F16/BF16 (1 cycle/row).
import os as _os

MM_DT = {"f32r": F32R, "f16": F16, "bf16": BF16}[_os.environ.get("MM_DT", "f32r")]

B, T, C = 4, 2048, 1024
H, D = 16, 64
HPC = 8          # heads per core
HD = HPC * D     # 512: per-core head-dim slab
N_CORES = 8
SCALE = D ** -0.5


def _np_of(dt):
    return np.dtype(mybir.dt.np(dt))


def build_nc(mm_dt=None):
    mm_dt = mm_dt or MM_DT
    nc = bacc.Bacc("TRN2", target_bir_lowering=False, debug=False)

    xT = nc.dram_tensor("xT", [C, T], mm_dt, kind="ExternalInput")
    wq = nc.dram_tensor("wq", [C, HD], mm_dt, kind="ExternalInput")
    wk = nc.dram_tensor("wk", [C, HD], mm_dt, kind="ExternalInput")
    wv = nc.dram_tensor("wv", [C, HD], mm_dt, kind="ExternalInput")
    wo = nc.dram_tensor("wo", [HD, C], mm_dt, kind="ExternalInput")
    # bq/bk as [128, HD//128] columns (per-partition adds in qkvT layout)
    bqc = nc.dram_tensor("bqc", [128, HD // 128], F32, kind="ExternalInput")
    bkc = nc.dram_tensor("bkc", [128, HD // 128], F32, kind="ExternalInput")
    bv = nc.dram_tensor("bv", [1, HD], mm_dt, kind="ExternalInput")
    tri = nc.dram_tensor("tri", [128, 128], mm_dt, kind="ExternalInput")
    out = nc.dram_tensor("out", [T, C], F32, kind="ExternalOutput")

    KO = C // 128        # 8 contraction tiles over C
    TC = T // 512        # 4 t-chunks of 512
    NQ = T // 512        # 4 query blocks per head
    NKT = T // 128       # 16 key tiles
    HDO = HD // 128      # 4 hd tiles

    with tile.TileContext(nc) as tc:
        const = tc.alloc_tile_pool(name="const", bufs=1)
        persist = tc.alloc_tile_pool(name="persist", bufs=1)
        psum = tc.alloc_tile_pool(name="psum", bufs=3, space="PSUM")
        psum_aug = tc.alloc_tile_pool(name="psum_aug", bufs=3, space="PSUM")
        psum_bc = tc.alloc_tile_pool(name="psum_bc", bufs=2, space="PSUM")

        # --- constants ---
        ones_f = const.tile([1, 512], F32)
        ones_r = const.tile([1, 512], mm_dt)
        nc.vector.memset(ones_f[:], 1.0)
        nc.vector.tensor_copy(ones_r[:], ones_f[:])
        ones_col_f = const.tile([128, 1], F32)
        nc.vector.memset(ones_col_f[:], 1.0)
        tri_sb = const.tile([128, 128], mm_dt)
        nc.sync.dma_start(tri_sb[:], tri[:, :])
        bqc_sb = const.tile([128, HD // 128], F32)
        bkc_sb = const.tile([128, HD // 128], F32)
        bv_sb = const.tile([1, HD], mm_dt)
        nc.sync.dma_start(bqc_sb[:], bqc[:, :])
        nc.sync.dma_start(bkc_sb[:], bkc[:, :])
        nc.sync.dma_start(bv_sb[:], bv[:, :])

        # --- persistent tensors ---
        qT_sb = persist.tile([128, HDO, T], mm_dt)   # [colpart, hd-outer, t]
        kT_sb = persist.tile([128, HDO, T], mm_dt)
        v_sb = persist.tile([128, NKT, HPC, D + 1], mm_dt)  # [tpart, ktile, head, d|1]
        nc.vector.tensor_copy(
            v_sb[:, :, :, D], ones_col_f[:, 0:1].to_broadcast([128, NKT, HPC])
        )

        # ---------------- Phase 1: qT, kT, v ----------------
        ph1 = tc.alloc_tile_pool(name="ph1", bufs=1)
        wq_sb = ph1.tile([128, KO, HD], mm_dt)
        wk_sb = ph1.tile([128, KO, HD], mm_dt)
        wv_sb = ph1.tile([128, KO, HD], mm_dt)
        nc.sync.dma_start(wq_sb[:], wq.rearrange("(ko p) m -> p ko m", p=128))
        nc.sync.dma_start(wk_sb[:], wk.rearrange("(ko p) m -> p ko m", p=128))
        nc.sync.dma_start(wv_sb[:], wv.rearrange("(ko p) m -> p ko m", p=128))
        xt_pool = tc.alloc_tile_pool(name="xt", bufs=9)

        for tc4 in range(TC):
            ts_ = slice(tc4 * 512, (tc4 + 1) * 512)
            xt = []
            for ko in range(KO):
                t_ = xt_pool.tile([128, 512], mm_dt, tag="xt")
                nc.sync.dma_start(t_[:], xT[ko * 128 : (ko + 1) * 128, ts_])
                xt.append(t_)
            # qT / kT column tiles: psum[colpart, t]; bias as per-partition add
            for w_sb, b_sb, dst in ((wq_sb, bqc_sb, qT_sb), (wk_sb, bkc_sb, kT_sb)):
                for i in range(HDO):
                    cs = slice(i * 128, (i + 1) * 128)
                    ps = psum.tile([128, 512], F32, tag="mm")
                    for ko in range(KO):
                        nc.tensor.matmul(
                            ps[:], w_sb[:, ko, cs], xt[ko][:],
                            start=(ko == 0), stop=(ko == KO - 1),
                        )
                    nc.vector.tensor_scalar_add(
                        dst[:, i, ts_], ps[:], b_sb[:, i : i + 1]
                    )
            # v tiles: psum[tpart, hd]; bias via K=1 rank-1 matmul
            for s in range(4):
                kt = tc4 * 4 + s
                ps = psum.tile([128, 512], F32, tag="mm")
                for ko in range(KO):
                    nc.tensor.matmul(
                        ps[:], xt[ko][:, s * 128 : (s + 1) * 128], wv_sb[:, ko, :],
                        start=(ko == 0), stop=False,
                    )
                nc.tensor.matmul(
                    ps[:], ones_r[0:1, 0:128], bv_sb[0:1, :],
                    start=False, stop=True, skip_group_check=True,
                )
                nc.scalar.copy(
                    v_sb[:, kt, :, 0:D], ps[:].rearrange("p (h d) -> p h d", h=HPC)
                )

        xt_pool.release()
        ph1.release()

        # ---------------- Phase 2: attention ----------------
        ph2 = tc.alloc_tile_pool(name="ph2", bufs=1)
        attnT_sb = ph2.tile([128, HDO, T], mm_dt)
        e_pool = tc.alloc_tile_pool(name="e", bufs=6)
        r_pool = tc.alloc_tile_pool(name="r", bufs=3)

        pending = None  # (aug, drow, pr, co, q) awaiting normalization

        def flush_norm():
            nonlocal pending
            if pending is None:
                return
            aug, drow, pr, co, q = pending
            bc = psum_bc.tile([64, 512], F32, tag="bc")
            nc.tensor.matmul(bc[:], ones_r[0:1, 0:64], drow[:], start=True, stop=True)
            rec = r_pool.tile([64, 512], F32, tag="rec")
            # ~4e-6 relerr, ~5x faster than exact reciprocal; denom >= ~0.04
            nc.vector.reciprocal_approx_fast(rec[:], bc[:])
            nc.vector.tensor_mul(
                attnT_sb[pr : pr + 64, co, q * 512 : (q + 1) * 512],
                aug[0:D, :], rec[:],
            )
            pending = None

        for h in range(HPC):
            co, pr = h // 2, (h % 2) * 64
            for q in range(NQ):
                jmax = 4 * q + 3
                aug = psum_aug.tile([D + 1, 512], F32, tag="aug")
                for j in range(jmax + 1):
                    diag = j >= 4 * q
                    c0 = 128 * (j - 4 * q) if diag else 0
                    ncol = 512 - c0
                    ps = psum.tile([128, 512], F32, tag="mm")
                    nc.tensor.matmul(
                        ps[:, :ncol],
                        kT_sb[pr : pr + 64, co, j * 128 : (j + 1) * 128],
                        qT_sb[pr : pr + 64, co, q * 512 + c0 : (q + 1) * 512],
                        start=True, stop=True,
                    )
                    e = e_pool.tile([128, 512], mm_dt, tag="e")
                    nc.scalar.activation(e[:, :ncol], ps[:, :ncol], EXP, scale=SCALE)
                    if diag:
                        nc.vector.tensor_mul(e[:, 0:128], e[:, 0:128], tri_sb[:])
                    nc.tensor.matmul(
                        aug[:, c0:], v_sb[:, j, h, :], e[:, :ncol],
                        start=(j == 0), stop=(j == jmax), skip_group_check=True,
                    )
                # denominator row out of PSUM, then normalize previous block
                drow = r_pool.tile([1, 512], mm_dt, tag="drow")
                with nc.allow_low_precision(reason="softmax denom rounding"):
                    nc.vector.tensor_copy(drow[:], aug[D : D + 1, :])
                flush_norm()
                pending = (aug, drow, pr, co, q)
        flush_norm()

        # ---------------- Phase 3: output projection ----------------
        ph3 = tc.alloc_tile_pool(name="ph3", bufs=1)
        wo_sb = ph3.tile([128, HDO, C], mm_dt)
        nc.sync.dma_start(wo_sb[:], wo.rearrange("(ko p) n -> p ko n", p=128))
        o_pool = tc.alloc_tile_pool(name="o", bufs=3)

        for tt in range(NKT):
            for cc in range(2):
                cs = slice(cc * 512, (cc + 1) * 512)
                ps = psum.tile([128, 512], F32, tag="mm")
                for ko in range(HDO):
                    nc.tensor.matmul(
                        ps[:], attnT_sb[:, ko, tt * 128 : (tt + 1) * 128],
                        wo_sb[:, ko, cs],
                        start=(ko == 0), stop=(ko == HDO - 1),
                    )
                osb = o_pool.tile([128, 512], F32, tag="osb")
                nc.vector.tensor_copy(osb[:], ps[:])
                nc.sync.dma_start(out[tt * 128 : (tt + 1) * 128, cs], osb[:])

        o_pool.release()
        ph3.release()
        r_pool.release()
        e_pool.release()
        ph2.release()
        psum_bc.release()
        psum_aug.release()
        psum.release()
        persist.release()
        const.release()

    nc.finalize()
    return nc


_NC_CACHE = {}


def _get_nc(mm_dt=None):
    key = str(mm_dt or MM_DT)
    if key not in _NC_CACHE:
        _NC_CACHE[key] = build_nc(mm_dt)
    return _NC_CACHE[key]


def make_in_maps(x, Wqkv, bqkv, Wo, mm_dt=None):
    mdt = _np_of(mm_dt or MM_DT)
    x = np.asarray(x, dtype=np.float32)
    Wqkv = np.asarray(Wqkv, dtype=np.float32)
    bqkv = np.asarray(bqkv, dtype=np.float32)
    Wo = np.asarray(Wo, dtype=np.float32)

    w3 = Wqkv.reshape(C, 3, H, D)
    b3 = bqkv.reshape(3, H, D)
    wo4 = Wo.reshape(H, D, C)
    tri = np.triu(np.ones((128, 128), dtype=np.float32))

    in_maps = []
    for c in range(N_CORES):
        b, g = c // 2, c % 2
        hs = slice(g * HPC, (g + 1) * HPC)
        bq = b3[0, hs].reshape(HD)
        bk = b3[1, hs].reshape(HD)
        in_maps.append({
            "xT": np.ascontiguousarray(x[b].T).astype(mdt),
            "wq": np.ascontiguousarray(w3[:, 0, hs, :].reshape(C, HD)).astype(mdt),
            "wk": np.ascontiguousarray(w3[:, 1, hs, :].reshape(C, HD)).astype(mdt),
            "wv": np.ascontiguousarray(w3[:, 2, hs, :].reshape(C, HD)).astype(mdt),
            "wo": np.ascontiguousarray(wo4[hs].reshape(HD, C)).astype(mdt),
            "bqc": np.ascontiguousarray(bq.reshape(HD // 128, 128).T).astype(np.float32),
            "bkc": np.ascontiguousarray(bk.reshape(HD // 128, 128).T).astype(np.float32),
            "bv": b3[2, hs].reshape(1, HD).astype(mdt),
            "tri": tri.astype(mdt),
        })
    return in_maps


def run(x, Wqkv, bqkv, Wo, bo, mm_dt=None, **spmd_kwargs):
    nc = _get_nc(mm_dt)
    in_maps = make_in_maps(x, Wqkv, bqkv, Wo, mm_dt=mm_dt)
    res = run_bass_kernel_spmd(nc, in_maps, core_ids=list(range(N_CORES)),
                               **spmd_kwargs)
    bo = np.asarray(bo, dtype=np.float32)
    out = np.empty((B, T, C), dtype=np.float32)
    for b in range(B):
        out[b] = res.results[2 * b]["out"] + res.results[2 * b + 1]["out"] + bo
    return out, res


def kernel(x, Wqkv, bqkv, Wo, bo):
    out, _ = run(x, Wqkv, bqkv, Wo, bo)
    return out
